# revision 2
# baseline (speedup 1.0000x reference)
"""2-layer GCN (GCNConv x2 + log_softmax) on 8 Trainium2 NeuronCores. v3.

Algorithm (unchanged from v2): separable norm folded into table rows /
output, both propagations at width 16, degree-ranked rounds giving
scatter-free segment sums, per-edge dma_gather with int16 idx, 4 chunk
tables (int16-range), chunk-q AllGather overlapped with gather work.

v3 performance changes, driven by HW microbenchmarks:
  * 4 SWDGE queues (num_swdge_queues=4). A single SWDGE queue drains
    ~28GB/s/core (descriptor-rate-bound, one DMA engine); 4 queues reach
    ~130-160GB/s/core. Gathers are round-robined across queues.
  * Queue assignment is aligned to the tile-assigned DMASW sem lane
    (queue = lane % 4, patched post-scheduling): each lane sem is then
    fed by exactly one queue, so lane sems stay monotonic in scheduled
    order -- mixing queues on one sem lane races on real HW (cross-queue
    completions reorder) even though per-gather data deps are satisfied.
  * Gathers read only the 16-f32 payload (elem_size=16, 64B) of each
    256B-strided table row instead of the full 256B row (the %256
    elem_size assert in bass.py is a transpose-mode restriction only;
    non-transpose ucode handles 64B elems -- validated exact on HW).
    Throughput is descriptor-rate-bound so the token rate is unchanged,
    but DRAM read pressure drops 4x and staging shrinks 4x.
  * Merge gathers (rank-order acc -> node order) use the same 64B path.
  * SBUF-source (transpose) dma_gather was evaluated and is broken on
    real HW when more than one SWDGE queue is active (exact at NQ=1);
    all gathers here are DRAM-source.
"""

import os
import sys
from contextlib import ExitStack

import numpy as np

if "/opt/trn_rl_repo" not in sys.path:
    sys.path.insert(0, "/opt/trn_rl_repo")

# ---------------------------------------------------------------- constants
N_NODES = 100000
NCORES = 8
F_IN = 512
HID = 16
NCLS = 40
P = 128
EP = 64          # table row pitch in f32 elems (256B, dma_gather minimum)
NCHUNK = 4       # table chunks (quarters of every core's range), int16-safe
G = int(os.environ.get("BASS_GCN_G", "2048"))      # tokens per gather
GBLK = int(os.environ.get("BASS_GCN_GBLK", "16"))  # gathers per idx block load
STGB = int(os.environ.get("BASS_GCN_STGB", "8"))   # staging pool depth
NQ = int(os.environ.get("BASS_GCN_NQ", "4"))       # SWDGE queues (max 4)
MERGE16 = os.environ.get("BASS_GCN_MERGE16", "1") == "1"
AGLATE = os.environ.get("BASS_GCN_AGLATE", "0") == "1"

LAST_EXEC_NS = None


def _dims():
    nloc = N_NODES // NCORES          # 12500
    tslot = -(-nloc // P)             # 98
    padloc = tslot * P                # 12544
    qrows = padloc // NCHUNK          # 3136 rows per core per chunk
    crows = NCORES * qrows            # 25088 rows per chunk table
    return nloc, tslot, padloc, qrows, crows


def _wrap16(flat):
    """int16 index vector -> [128, n/16] dma_gather layout (16-partition
    wrap, replicated 8x down the partitions)."""
    n = flat.size
    assert n % 16 == 0
    blk = flat.reshape(n // 16, 16).T
    return np.tile(blk, (8, 1)).astype(np.int16)


def _l_of_pos():
    """On-chip position pos = 128*t + p holds local node L_of_pos[pos].

    Within each 128-node tile the nodes are permuted so that quarter
    q = l%4 occupies partitions [32q, 32q+32): p = (l%4)*32 + (l%128)//4.
    This makes each quarter's table dump a contiguous partition slice.
    """
    _, tslot, padloc, _, _ = _dims()
    pos = np.arange(padloc)
    t, p = pos // P, pos % P
    return t * P + (p % 32) * 4 + p // 32      # inverse of the p-mapping


def _pos_of_l():
    _, tslot, padloc, _, _ = _dims()
    lop = _l_of_pos()
    pol = np.empty(padloc, np.int64)
    pol[lop] = np.arange(padloc)
    return pol


# ================================================================ host plan
def _plan(edge_index):
    nloc, tslot, padloc, qrows, crows = _dims()
    assert crows - 1 <= np.iinfo(np.int16).max
    # chunk q of global node g = (g % nloc) % NCHUNK (interleaved quarters);
    # row within chunk table: owner(g)*qrows + (g % nloc)//NCHUNK.
    # Rows [nloc//NCHUNK, qrows) of core 0's stripe are dump-zeroed pads:
    pad_row = nloc // NCHUNK          # 3125: zero row usable as pad target

    src = np.asarray(edge_index[0]).astype(np.int64)
    dst = np.asarray(edge_index[1]).astype(np.int64)
    owner = dst // nloc

    entries = [[None] * NCHUNK for _ in range(NCORES)]
    nrounds = np.zeros((NCORES, NCHUNK), np.int64)
    merge_idx_flat = np.empty((NCORES, NCHUNK, padloc), np.int64)
    deg_node = np.full((NCORES, P, tslot), 1e38, np.float32)

    for c in range(NCORES):
        m = owner == c
        s_c = src[m]
        l_c = dst[m] - c * nloc
        deg_tot = np.bincount(l_c, minlength=nloc)
        dn = np.full(padloc, 1e38, np.float32)
        dn[:nloc] = deg_tot + 1.0
        deg_node[c] = dn[_l_of_pos()].reshape(tslot, P).T

        s_l = s_c % nloc
        cq = s_l % NCHUNK
        s_row = (s_c // nloc) * qrows + s_l // NCHUNK
        for q in range(NCHUNK):
            mq = cq == q
            r_q, l_q = s_row[mq], l_c[mq]
            deg_q = np.bincount(l_q, minlength=nloc)
            order_q = np.argsort(-deg_q, kind="stable")
            rank_of = np.empty(nloc, np.int64)
            rank_of[order_q] = np.arange(nloc)
            r_e = rank_of[l_q]
            o1 = np.argsort(r_e, kind="stable")
            rs, rows1 = r_e[o1], r_q[o1]
            deg_rank = deg_q[order_q]
            starts = np.zeros(nloc, np.int64)
            starts[1:] = np.cumsum(deg_rank)[:-1]
            occ = np.arange(rs.size, dtype=np.int64) - starts[rs]
            o2 = np.argsort(occ * nloc + rs, kind="stable")
            rows_sorted = rows1[o2].astype(np.int64)
            n_r = (np.bincount(occ).astype(np.int64) if occ.size
                   else np.zeros(0, np.int64))
            offs = np.zeros(n_r.size + 1, np.int64)
            offs[1:] = np.cumsum(n_r)
            entries[c][q] = (rows_sorted, n_r, offs)
            nrounds[c, q] = n_r.size

            mi = np.full(padloc, padloc - 1, np.int64)  # dead (zero) acc slot
            lop = _l_of_pos()
            valid = lop < nloc
            mi[valid] = rank_of[lop[valid]]
            merge_idx_flat[c, q] = mi

    # shared round sizes (max over cores) -> per-chunk token tapes
    tapes = []        # per chunk: list of (round_start, n_r) in tape tokens
    tape_len = []
    for q in range(NCHUNK):
        rmax = int(nrounds[:, q].max())
        nm = np.zeros(rmax, np.int64)
        for c in range(NCORES):
            n_r = entries[c][q][1]
            nm[: n_r.size] = np.maximum(nm[: n_r.size], n_r)
        rounds, pos = [], 0
        for n in nm:
            n = int(n)
            if n <= 0:
                continue
            rounds.append((pos, n))
            pos += (-(-n // P)) * P
        tapes.append(rounds)
        tape_len.append(pos)

    # per-core gather index tapes
    gidx = []
    for q in range(NCHUNK):
        arr = np.full((NCORES, tape_len[q]), pad_row, np.int64)
        for c in range(NCORES):
            rows_sorted, n_r, offs = entries[c][q]
            for r, (pos, n) in enumerate(tapes[q]):
                if r < n_r.size:
                    v = rows_sorted[offs[r]: offs[r + 1]]
                    arr[c, pos: pos + v.size] = v
        gidx.append(np.stack([_wrap16(arr[c].astype(np.int16))
                              for c in range(NCORES)]))

    midx = np.empty((NCORES, NCHUNK, P, padloc // 16), np.int16)
    for c in range(NCORES):
        for q in range(NCHUNK):
            midx[c, q] = _wrap16(merge_idx_flat[c, q].astype(np.int16))

    meta = dict(tapes=tapes, tape_len=tape_len, tslot=tslot, nloc=nloc,
                padloc=padloc, qrows=qrows, crows=crows, pad_row=pad_row)
    host = dict(gidx=gidx, midx=midx, deg_node=deg_node)
    return meta, host


def _raw_gather(ncx, out_ap, in_ap, idxs_ap, ni, elem, queue_num=0):
    """dma_gather with elem_size=elem f32 at 256B row stride; elem=16
    bypasses bass.py's %256 elem assert (non-transpose ucode allows it)."""
    from concourse import mybir
    gp = ncx.gpsimd
    _in_ap = gp.lower_ap_dma(in_ap, for_custom_bir_dma=True)
    _idxs_ap = gp.lower_ap(idxs_ap)
    _out_ap = gp.lower_ap(out_ap)
    return gp.add_instruction(
        mybir.InstDMAGatherAnt(
            name=gp.bass.get_next_instruction_name(),
            ins=[*_in_ap, _idxs_ap, gp.lower_val_access(gp.to_reg(ni))],
            outs=[_out_ap],
            transpose=False,
            num_idxs=ni,
            elem_size=elem,
            stride_bytes_256=1,
            gen_mode=0,
            single_packet=False,
            queue_num=queue_num,
        )
    )


# ============================================================ device program
def _emit(tc, io, meta, reps=1):
    import concourse.tile as tile  # noqa: F401
    from concourse import mybir

    nc = tc.nc
    f32 = mybir.dt.float32
    i16 = mybir.dt.int16
    nloc, tslot, padloc = meta["nloc"], meta["tslot"], meta["padloc"]
    qrows, crows = meta["qrows"], meta["crows"]
    tapes, tape_len = meta["tapes"], meta["tape_len"]
    kch = F_IN // P
    AF = mybir.ActivationFunctionType
    AL = mybir.AluOpType
    ABLATE = os.environ.get("BASS_GCN_ABLATE", "")
    gslot = G // P

    with ExitStack() as ctx:
        sb = ctx.enter_context(tc.tile_pool(name="sb", bufs=1))
        xb = ctx.enter_context(tc.tile_pool(name="xb", bufs=3))
        stg = ctx.enter_context(tc.tile_pool(name="stg", bufs=STGB))
        ib = ctx.enter_context(tc.tile_pool(name="ib", bufs=3))
        tp = ctx.enter_context(tc.tile_pool(name="tp", bufs=2))
        totp = ctx.enter_context(tc.tile_pool(name="totp", bufs=1))
        ps = ctx.enter_context(tc.tile_pool(name="ps", bufs=2, space="PSUM"))
        ps1 = ctx.enter_context(tc.tile_pool(name="ps1", bufs=1, space="PSUM"))
        dram = ctx.enter_context(tc.tile_pool(name="dram", bufs=1, space="DRAM"))

        # ---- persistent small tiles
        w1t = sb.tile([P, kch * HID], f32, tag="w1")
        nc.sync.dma_start(
            w1t[:].rearrange("p (k h) -> p k h", h=HID),
            io["W1"].rearrange("(k p) h -> p k h", p=P),
        )
        w2t = sb.tile([HID, NCLS], f32, tag="w2")
        nc.sync.dma_start(w2t[:], io["W2"])

        ones1 = sb.tile([1, P], f32, tag="ones1")
        nc.vector.memset(ones1[:], 1.0)
        b1s = sb.tile([1, HID], f32, tag="b1s")
        nc.sync.dma_start(b1s[:], io["b1"])
        b2s = sb.tile([1, NCLS], f32, tag="b2s")
        nc.sync.dma_start(b2s[:], io["b2"])
        b1p = ps1.tile([P, HID], f32, tag="biasp")
        nc.tensor.matmul(b1p[:], lhsT=ones1[:], rhs=b1s[:], start=True, stop=True)
        b1bc = sb.tile([P, HID], f32, tag="b1bc")
        nc.vector.tensor_copy(b1bc[:], b1p[:])
        b2p = ps1.tile([P, NCLS], f32, tag="biasp")
        nc.tensor.matmul(b2p[:], lhsT=ones1[:], rhs=b2s[:], start=True, stop=True)
        b2bc = sb.tile([P, NCLS], f32, tag="b2bc")
        nc.vector.tensor_copy(b2bc[:], b2p[:])

        zrow = sb.tile([P, EP], f32, tag="zrow")
        nc.vector.memset(zrow[:], 0.0)

        dn0 = sb.tile([P, tslot], f32, tag="dn0")
        nc.sync.dma_start(dn0[:], io["deg_node"])
        dn1 = sb.tile([P, tslot], f32, tag="dn1")
        nc.vector.reciprocal(dn1[:], dn0[:])
        disn = sb.tile([P, tslot], f32, tag="disn")
        nc.scalar.activation(disn[:], dn1[:], AF.Sqrt)
        disn3h = disn[:].unsqueeze(2).to_broadcast([P, tslot, HID])

        ident = sb.tile([P, P], f32, tag="ident")
        from concourse.masks import make_identity
        make_identity(nc, ident[:])

        # merge index tiles (persistent, shared by both layers)
        mits = []
        for q in range(NCHUNK):
            mit = sb.tile([P, padloc // 16], i16, tag=f"mit{q}")
            nc.sync.dma_start(mit[:], io["midx"][q])
            mits.append(mit)

        for _rep in range(reps):
            # ---- phase A: h_hat = dis_node * (x @ W1)   (node-tile layout)
            hh = sb.tile([P, tslot * HID], f32, tag="hh")
            nc.vector.memset(hh[:], 0.0)
            for t in ([] if "noA" in ABLATE else range(tslot)):
                xt = xb.tile([P, kch * P], f32, tag="xt")
                nc.sync.dma_start(
                    xt[:].rearrange("p (k n) -> p k n", k=kch),
                    io["xT"][:, t * P: (t + 1) * P].rearrange("(k p) n -> p k n",
                                                             p=P),
                )
                hp = ps.tile([P, HID], f32, tag="hp")
                for k in range(kch):
                    nc.tensor.matmul(
                        hp[:, :],
                        lhsT=xt[:, k * P: (k + 1) * P],
                        rhs=w1t[:, k * HID: (k + 1) * HID],
                        start=(k == 0),
                        stop=(k == kch - 1),
                    )
                nc.vector.tensor_scalar_mul(
                    hh[:, t * HID: (t + 1) * HID], hp[:, :], disn[:, t: t + 1]
                )

            def dump_table(src16, dst_qs):
                """[128, tslot*16] SBUF -> NCHUNK x [qrows, EP] quarter dumps.

                With the within-tile node permutation, quarter q lives on
                partitions [32q, 32q+32); node l -> dump row l//4 = 32t+w."""
                W = P // NCHUNK
                for q in range(NCHUNK):
                    nc.sync.dma_start(
                        dst_qs[q][:].rearrange("(t w) e -> w t e", w=W)
                        [:, :, :HID],
                        src16[32 * q: 32 * (q + 1), :].rearrange(
                            "w (t h) -> w t h", h=HID),
                    )

            hhd = [dram.tile([qrows, EP], f32, tag=f"hhd{q}", name=f"hhd{q}")
                   for q in range(NCHUNK)]
            ztd = [dram.tile([qrows, EP], f32, tag=f"ztd{q}", name=f"ztd{q}")
                   for q in range(NCHUNK)]
            accd = [dram.tile([padloc, EP], f32, tag=f"accd{q}", name=f"accd{q}")
                    for q in range(NCHUNK)]
            adsp = "Local" if "noag" in ABLATE else "Shared"
            tabs1 = [dram.tile([crows, EP], f32, tag=f"tab1_{q}",
                               name=f"tab1_{q}", addr_space=adsp)
                     for q in range(NCHUNK)]
            tabs2 = [dram.tile([crows, EP], f32, tag=f"tab2_{q}",
                               name=f"tab2_{q}", addr_space=adsp)
                     for q in range(NCHUNK)]

            def ag_one(local_qs, tabs, q):
                if "noag" in ABLATE:
                    for c in range(NCORES):
                        nc.sync.dma_start(
                            tabs[q][c * qrows: (c + 1) * qrows, :],
                            local_qs[q][:, :])
                    return
                nc.gpsimd.collective_compute(
                    "AllGather", AL.bypass,
                    replica_groups=[list(range(NCORES))],
                    ins=[local_qs[q][:, :].opt()],
                    outs=[tabs[q][:, :].opt()],
                )

            # ---- tape-cut gather + segment-sum + merge
            def propagate(tabs, self16, layer, ag=None):
                accs = []
                for q in range(NCHUNK):
                    a = tp.tile([P, tslot * HID], f32, tag=f"acc{q}")
                    nc.vector.memset(a[:], 0.0)
                    accs.append(a)
                tot = totp.tile([P, tslot * HID], f32, tag=f"tot{layer}")
                nc.vector.tensor_copy(tot[:], self16[:])
                tot3 = tot[:].rearrange("p (t h) -> p t h", h=HID)

                for q in ([] if "nogather" in ABLATE else range(NCHUNK)):
                    if ag is not None:
                        ag(q)
                    a3 = accs[q][:].rearrange("p (t h) -> p t h", h=HID)
                    L = tape_len[q]
                    ngath = -(-L // G)
                    for blk0 in range(0, ngath, GBLK):
                        blk1 = min(blk0 + GBLK, ngath)
                        cols0 = blk0 * (G // 16)
                        cols1 = min(blk1 * (G // 16), L // 16)
                        it = ib.tile([P, cols1 - cols0], i16, tag="it")
                        nc.sync.dma_start(it[:], io[f"gidx{q}"][:, cols0:cols1])
                        for g in range(blk0, blk1):
                            a0 = g * G
                            ni = min(G, L - a0)
                            st = stg.tile([P, gslot * HID], f32, tag="st16")
                            st3 = st[:].rearrange("p (s e) -> p s e", e=HID)
                            _raw_gather(
                                nc, st3[:, : ni // P, :],
                                tabs[q][:, :HID],
                                it[:, g * (G // 16) - cols0:
                                   g * (G // 16) - cols0 + ni // 16],
                                ni, HID)
                            # adds: rounds overlapping tape window [a0, a0+ni)
                            for (pos, n) in tapes[q]:
                                lo = max(pos, a0)
                                hi = min(pos + n, a0 + ni)
                                if lo >= hi:
                                    continue
                                # full slots then remainder
                                s0 = (lo - a0) // P        # staging slot
                                t0 = (lo - pos) // P       # acc slot
                                nfull = (hi - lo) // P
                                rem = (hi - lo) % P
                                if nfull:
                                    nc.vector.tensor_add(
                                        a3[:, t0: t0 + nfull, :],
                                        a3[:, t0: t0 + nfull, :],
                                        st3[:, s0: s0 + nfull, :])
                                if rem:
                                    nc.vector.tensor_add(
                                        a3[:rem, t0 + nfull: t0 + nfull + 1, :],
                                        a3[:rem, t0 + nfull: t0 + nfull + 1, :],
                                        st3[:rem, s0 + nfull: s0 + nfull + 1,
                                            :])
                    # merge chunk q back to node order (tape-cut too)
                    if "nomerge" in ABLATE:
                        continue
                    dump_q = accd[q]
                    if "nomg" in ABLATE:
                        nc.sync.dma_start(
                            dump_q[:].rearrange("(t p) e -> p t e", p=P)
                            [:, :, :HID],
                            accs[q][:].rearrange("p (t h) -> p t h", h=HID),
                        )
                        continue
                    nc.sync.dma_start(
                        dump_q[:].rearrange("(t p) e -> p t e", p=P)[:, :, :HID],
                        accs[q][:].rearrange("p (t h) -> p t h", h=HID),
                    )
                    for a0 in range(0, padloc, G):
                        ni = min(G, padloc - a0)
                        if MERGE16:
                            mst = stg.tile([P, gslot * HID], f32, tag="st16")
                            mst3 = mst[:].rearrange("p (s e) -> p s e", e=HID)
                            _raw_gather(
                                nc, mst3[:, : ni // P, :],
                                dump_q[:, :HID],
                                mits[q][:, a0 // 16: (a0 + ni) // 16],
                                ni, HID)
                            nc.vector.tensor_add(
                                tot3[:, a0 // P: (a0 + ni) // P, :],
                                tot3[:, a0 // P: (a0 + ni) // P, :],
                                mst3[:, : ni // P, :])
                        else:
                            mst = stg.tile([P, gslot * EP], f32, tag="st")
                            mst3 = mst[:].rearrange("p (s e) -> p s e", e=EP)
                            nc.gpsimd.dma_gather(
                                out_ap=mst3[:, : ni // P, :],
                                in_ap=dump_q[:, :],
                                idxs_ap=mits[q][:, a0 // 16: (a0 + ni) // 16],
                                num_idxs=ni,
                                num_idxs_reg=ni,
                                elem_size=EP,
                                single_packet=False,
                            )
                            nc.vector.tensor_add(
                                tot3[:, a0 // P: (a0 + ni) // P, :],
                                tot3[:, a0 // P: (a0 + ni) // P, :],
                                mst3[:, : ni // P, :HID])
                return tot

            dump_table(hh, hhd)
            if AGLATE:
                tot1 = propagate(tabs1, hh, 1,
                                 ag=lambda q: ag_one(hhd, tabs1, q))
            else:
                for q in range(NCHUNK):
                    ag_one(hhd, tabs1, q)
                tot1 = propagate(tabs1, hh, 1)

            # ---- z_hat = dis * relu(dis * tot1 + b1)   (node order)
            zt = sb.tile([P, tslot * HID], f32, tag="zt")
            zt3 = zt[:].rearrange("p (t h) -> p t h", h=HID)
            tot13 = tot1[:].rearrange("p (t h) -> p t h", h=HID)
            nc.vector.tensor_tensor(zt3, tot13, disn3h, op=AL.mult)
            nc.vector.tensor_tensor(
                zt3, zt3, b1bc[:].unsqueeze(1).to_broadcast([P, tslot, HID]),
                op=AL.add)
            nc.scalar.activation(zt[:], zt[:], AF.Relu)
            nc.vector.tensor_tensor(zt3, zt3, disn3h, op=AL.mult)

            dump_table(zt, ztd)
            if AGLATE:
                tot2 = propagate(tabs2, zt, 2,
                                 ag=lambda q: ag_one(ztd, tabs2, q))
            else:
                for q in range(NCHUNK):
                    ag_one(ztd, tabs2, q)
                tot2 = propagate(tabs2, zt, 2)

            # ---- p = dis * tot2 ; logits = p @ W2 + b2 ; log_softmax
            pf = sb.tile([P, tslot * HID], f32, tag="pf")
            pf3 = pf[:].rearrange("p (t h) -> p t h", h=HID)
            nc.vector.tensor_tensor(
                pf3, tot2[:].rearrange("p (t h) -> p t h", h=HID), disn3h,
                op=AL.mult)

            lg = sb.tile([P, tslot * NCLS], f32, tag="lg")
            for t in range(tslot):
                ptp = ps.tile([HID, P], f32, tag="ptp")
                nc.tensor.transpose(ptp[:], pf[:, t * HID: (t + 1) * HID], ident[:])
                pts = tp.tile([HID, P], f32, tag="pts")
                nc.vector.tensor_copy(pts[:], ptp[:])
                lp = ps.tile([P, NCLS], f32, tag="lp")
                nc.tensor.matmul(lp[:], lhsT=pts[:], rhs=w2t[:], start=True, stop=True)
                nc.vector.tensor_add(lg[:, t * NCLS: (t + 1) * NCLS], lp[:], b2bc[:])

            lg3 = lg[:].rearrange("p (t c) -> p t c", c=NCLS)
            mx = sb.tile([P, tslot], f32, tag="mx")
            nc.vector.reduce_max(out=mx[:], in_=lg3, axis=mybir.AxisListType.X)
            nc.vector.tensor_tensor(
                lg3, lg3, mx[:].unsqueeze(2).to_broadcast([P, tslot, NCLS]),
                op=AL.subtract)
            ex = sb.tile([P, tslot * NCLS], f32, tag="ex")
            nc.scalar.activation(ex[:], lg[:], AF.Exp)
            sm = sb.tile([P, tslot], f32, tag="sm")
            nc.vector.reduce_sum(
                out=sm[:], in_=ex[:].rearrange("p (t c) -> p t c", c=NCLS),
                axis=mybir.AxisListType.X)
            ls = sb.tile([P, tslot], f32, tag="ls")
            nc.scalar.activation(ls[:], sm[:], AF.Ln)
            nc.vector.tensor_tensor(
                lg3, lg3, ls[:].unsqueeze(2).to_broadcast([P, tslot, NCLS]),
                op=AL.subtract)
            nc.sync.dma_start(io["out_raw"], lg[:])


def build_nc(meta, reps=1):
    import concourse.bacc as bacc
    import concourse.tile as tile
    from concourse import mybir

    nloc, tslot, padloc, qrows, crows = _dims()
    f32, i16 = mybir.dt.float32, mybir.dt.int16

    nc = bacc.Bacc("TRN2", target_bir_lowering=False, debug=False,
                   num_devices=NCORES, num_swdge_queues=NQ)
    io = {
        "xT": nc.dram_tensor("xT", [F_IN, padloc], f32, kind="ExternalInput").ap(),
        "W1": nc.dram_tensor("W1", [F_IN, HID], f32, kind="ExternalInput").ap(),
        "b1": nc.dram_tensor("b1", [1, HID], f32, kind="ExternalInput").ap(),
        "W2": nc.dram_tensor("W2", [HID, NCLS], f32, kind="ExternalInput").ap(),
        "b2": nc.dram_tensor("b2", [1, NCLS], f32, kind="ExternalInput").ap(),
        "deg_node": nc.dram_tensor("deg_node", [P, tslot], f32,
                                   kind="ExternalInput").ap(),
        "midx": nc.dram_tensor("midx", [NCHUNK, P, padloc // 16], i16,
                               kind="ExternalInput").ap(),
        "out_raw": nc.dram_tensor("out_raw", [P, tslot * NCLS], f32,
                                  kind="ExternalOutput").ap(),
    }
    for q in range(NCHUNK):
        io[f"gidx{q}"] = nc.dram_tensor(
            f"gidx{q}", [P, meta["tape_len"][q] // 16], i16,
            kind="ExternalInput").ap()
    with tile.TileContext(nc) as tc:
        _emit(tc, io, meta, reps=reps)
    if NQ > 1:
        # Align each SWDGE gather's queue with its tile-assigned DMASW sem
        # lane (lanes are handed out round-robin in scheduled order); lane
        # i -> queue i%NQ keeps every sem fed from a single queue, so lane
        # sems stay monotonic in scheduled order (no cross-queue races).
        from concourse.tile_sem_assignment import PROC_NAME_TO_IDX
        lane_of_proc = {PROC_NAME_TO_IDX[f"DMASW{i}"]: i for i in range(8)}
        for bb in nc.main_func.blocks:
            for inst in bb.instructions:
                if isinstance(inst, mybir.InstDMAGatherAnt):
                    lane = lane_of_proc.get(
                        getattr(inst, "bass_scheduled_proc", None))
                    if lane is not None:
                        inst.queue_num = lane % NQ
    nc.compile()
    return nc


def make_in_maps(inputs, host):
    nloc, _, padloc, _, _ = _dims()
    x = np.asarray(inputs["x"], np.float32)
    lop = _l_of_pos()
    W1 = np.ascontiguousarray(np.asarray(inputs["W1"], np.float32))
    b1 = np.asarray(inputs["b1"], np.float32).reshape(1, HID)
    W2 = np.ascontiguousarray(np.asarray(inputs["W2"], np.float32))
    b2 = np.asarray(inputs["b2"], np.float32).reshape(1, NCLS)
    in_maps = []
    for c in range(NCORES):
        xp = np.zeros((padloc, F_IN), np.float32)
        valid = lop < nloc
        xp[valid] = x[c * nloc + lop[valid]]
        im = {
            "xT": np.ascontiguousarray(xp.T),
            "W1": W1, "b1": b1, "W2": W2, "b2": b2,
            "deg_node": host["deg_node"][c],
            "midx": host["midx"][c],
        }
        for q in range(NCHUNK):
            im[f"gidx{q}"] = host["gidx"][q][c]
        in_maps.append(im)
    return in_maps


def unshard(results, host):
    nloc, tslot, padloc, _, _ = _dims()
    pol = _pos_of_l()
    out = np.empty((N_NODES, NCLS), np.float32)
    for c in range(NCORES):
        raw = results[c]["out_raw"].reshape(P, tslot, NCLS)
        flat = raw.transpose(1, 0, 2).reshape(padloc, NCLS)
        out[c * nloc: (c + 1) * nloc] = flat[pol[:nloc]]
    return out


def run_hw_timed_pair(nc1, nch, in_maps, warmup=6, iters=12):
    """Interleaved timing of two NEFFs (reps=1 vs reps=hi) with warmup, so
    DVFS/clock state is identical for both; returns (res1, med_t1, med_th)."""
    import time

    import numpy

    run1, res1_fn = _make_hw_runner(nc1, in_maps)
    runh, _ = _make_hw_runner(nch, in_maps)
    for _ in range(warmup):
        run1()
        runh()
    t1s, ths = [], []
    out1 = None
    for _ in range(iters):
        t0 = time.perf_counter()
        out1 = run1()
        t1s.append(time.perf_counter() - t0)
        t0 = time.perf_counter()
        runh()
        ths.append(time.perf_counter() - t0)
    return res1_fn(out1), float(numpy.median(t1s)), float(numpy.median(ths))


def _make_hw_runner(nc, in_maps):
    """jit'd device-resident runner for one NEFF; returns (run, results_of)."""
    import jax
    from concourse import bass2jax, mybir

    bass2jax.install_neuronx_cc_hook()
    partition_name = (nc.partition_id_tensor.name
                      if nc.partition_id_tensor else None)
    in_names, out_names, out_avals, zero_outs = [], [], [], []
    for alloc in nc.m.functions[0].allocations:
        if not isinstance(alloc, mybir.MemoryLocationSet):
            continue
        name = alloc.memorylocations[0].name
        if alloc.kind == "ExternalInput":
            if name != partition_name:
                in_names.append(name)
        elif alloc.kind == "ExternalOutput":
            out_names.append(name)
            shape = tuple(alloc.tensor_shape)
            dtype = mybir.dt.np(alloc.dtype)
            out_avals.append(jax.core.ShapedArray(shape, dtype))
            zero_outs.append(np.zeros(shape, dtype))
    n_params = len(in_names)
    all_names = in_names + out_names
    if partition_name is not None:
        all_names = all_names + [partition_name]

    def _body(*args):
        operands = list(args)
        if partition_name is not None:
            operands.append(bass2jax.partition_id_tensor())
        outs = bass2jax._bass_exec_p.bind(
            *operands,
            out_avals=tuple(out_avals),
            in_names=tuple(all_names),
            out_names=tuple(out_names),
            lowering_input_output_aliases=(),
            sim_require_finite=True,
            sim_require_nnan=True,
            nc=nc,
        )
        return tuple(outs)

    devices = jax.devices()[:NCORES]
    mesh = bass2jax.Mesh(np.asarray(devices), ("core",))
    pspec = bass2jax.PartitionSpec("core")
    sharded = jax.jit(
        bass2jax.shard_map(_body, mesh=mesh,
                           in_specs=(pspec,) * (n_params + len(out_names)),
                           out_specs=(pspec,) * len(out_names),
                           check_rep=False),
        keep_unused=True,
    )
    sharding = jax.sharding.NamedSharding(mesh, pspec)
    dev_in = [
        jax.device_put(
            np.concatenate([np.asarray(in_maps[c][n]) for c in range(NCORES)],
                           axis=0), sharding)
        for n in in_names
    ]
    dev_zero = [
        jax.device_put(np.zeros((NCORES * z.shape[0], *z.shape[1:]), z.dtype),
                       sharding)
        for z in zero_outs
    ]
    jax.block_until_ready(dev_in + dev_zero)

    def run():
        out = sharded(*dev_in, *dev_zero)
        jax.block_until_ready(out)
        return out

    def results_of(out_arrs):
        return [
            {name: np.asarray(out_arrs[i]).reshape(NCORES, *out_avals[i].shape)[c]
             for i, name in enumerate(out_names)}
            for c in range(NCORES)
        ]

    return run, results_of


def kernel(**inputs):
    global LAST_EXEC_NS
    from concourse.bass_utils import run_bass_kernel_spmd

    meta, host = _plan(np.asarray(inputs["edge_index"]))
    nc = build_nc(meta)
    in_maps = make_in_maps(inputs, host)
    res = run_bass_kernel_spmd(nc, in_maps, core_ids=list(range(NCORES)))
    LAST_EXEC_NS = res.exec_time_ns
    return unshard(res.results, host)

